# revision 21
# baseline (speedup 1.0000x reference)
"""Causal multi-head attention (B=4, T=2048, D=512, H=8) on 8 TRN2 NeuronCores.

Sharding: core c handles batch b = c//2 and heads [4*(c%2), 4*(c%2)+4).
Data parallel on B (4 batches x 2 cores each), tensor parallel on H
(w_qkv column-sharded, w_proj row-sharded). Each core produces a partial
projection output yT [512, 2048] (f16); the host sums the two partials per
batch, transposes, and adds b_proj.

Schedule: the 80 (pair, q-chunk, s-block) attention blocks form one
software-pipelined stream paced by the scalar engine's exp (~1.1us/block).
Per step t:
  PE : QK(t+1) -> AV(t) -> deadline-scheduled filler matmuls (qkv/proj)
  ACT: exp(t+1)   (back-to-back)
  DVE: psum->sbuf copies + masks; normalize chains deferred to the pc=3
       region where DVE has slack (only the oc copy is urgent: it frees
       the PSUM accumulator bank in ~0.7us for the next segment)
Inputs arrive host-permuted partition-major; first-needed bytes sit at the
head of each DMA queue (SDMA engines round-robin queues per packet but are
FIFO within one). The last segment's accumulator is split into column
halves in separate banks: by causality the low half is final two s-blocks
early, so its normalize+projection hide under the end of the exp stream.
PSUM: 2x2-bank scores + 2x1-bank accumulators + 2x1-bank staging.
"""

import numpy as np

import concourse.bass as bass
import concourse.mybir as mybir
from concourse.bacc import Bacc
from concourse.tile import TileContext

F16 = mybir.dt.float16
F32 = mybir.dt.float32
FT = mybir.ActivationFunctionType
OP = mybir.AluOpType

T = 2048
D = 512
HPC = 4  # heads per core
K = 64  # head dim
P = 128
NSB = T // P  # 16 s-blocks


def build_nc():
    nc = Bacc()

    # all inputs host-permuted to [128 partitions, contiguous-per-partition]
    xTc = [
        nc.declare_dram_parameter(f"xT{ch}", [P, 2048], F16, isOutput=False)
        for ch in range(4)
    ]
    wq = nc.declare_dram_parameter("wq", [P, 1024], F16, isOutput=False)
    wk = nc.declare_dram_parameter("wk", [P, 1024], F16, isOutput=False)
    wv = nc.declare_dram_parameter("wv", [P, 1024], F16, isOutput=False)
    wp = nc.declare_dram_parameter("wp", [P, 1024], F16, isOutput=False)
    yT = nc.declare_dram_parameter("yT", [P, 4 * T], F16, isOutput=True)

    yT_dram = yT.rearrange("p (m t) -> p m t", t=T)

    with TileContext(nc) as tc:
        with (
            tc.tile_pool(name="persist", bufs=1) as pp,
            tc.tile_pool(name="ex", bufs=6) as ex_pool,
            tc.tile_pool(name="nrm", bufs=3) as nrm_pool,
            tc.tile_pool(name="ocp", bufs=8) as oc_pool,
            tc.tile_pool(name="sc", bufs=2, space="PSUM") as sc_pool,
            tc.tile_pool(name="outs", bufs=2, space="PSUM") as outs_pool,
            tc.tile_pool(name="stg", bufs=2, space="PSUM") as stg_pool,
        ):
            # ---- persistent SBUF tensors ----
            xT_sb = [
                pp.tile([P, 4, 4, 128], F16, tag=f"xT_sb{ch}", name=f"xT_sb{ch}")
                for ch in range(4)
            ]
            qT_sb = pp.tile([P, 2, T], F16, tag="qT_sb")
            kT_sb = pp.tile([P, 2, T], F16, tag="kT_sb")
            # per t-block, head h at cols 128h: [v (64) | ones | zero pad] so
            # AV's stationary operand is a full 128 columns (FWL + full PE)
            v_sb = pp.tile([P, NSB, HPC * P], F16, tag="v_sb")
            zn_sb = pp.tile([P, 2, T], F16, tag="zn_sb")
            yT_sb = pp.tile([P, 4, T], F16, tag="yT_sb")
            wq_sb = pp.tile([P, 4, 256], F16, tag="wq_sb")
            wk_sb = pp.tile([P, 4, 256], F16, tag="wk_sb")
            wv_sb = pp.tile([P, 4, 256], F16, tag="wv_sb")
            wp_sb = pp.tile([P, 2, D], F16, tag="wp_sb")
            trimask = pp.tile([P, 2, P], F16, tag="trimask")
            warm_sb = pp.tile([P, 512], F16, tag="warm_sb")
            junk_sb = pp.tile([P, P], F16, tag="junk_sb")

            # ---- constants (gpsimd builds trimask; DVE zeroes v pad) ----
            nc.gpsimd.memset(warm_sb[:], 0.0)
            # trimask[p, b, f] = 1 if f >= p else 0 (two copies, one per head)
            nc.gpsimd.memset(trimask[:], 1.0)
            for b2 in range(2):
                nc.gpsimd.affine_select(
                    out=trimask[:, b2, :],
                    in_=trimask[:, b2, :],
                    compare_op=OP.is_ge,
                    fill=0.0,
                    base=0,
                    pattern=[[1, P]],
                    channel_multiplier=-1,
                )
            for h in range(HPC):
                nc.vector.memset(v_sb[:, :, h * P + K + 1 : (h + 1) * P], 0.0)
                nc.vector.memset(v_sb[:, :, h * P + K : h * P + K + 1], 1.0)

            # ---- input DMAs. Issues cost ~0.8us of engine time each, and
            # each SDMA engine is FIFO within a queue: put first-needed bytes
            # at each queue head; keep the scalar (ACT) ring nearly free so
            # the exp stream is not delayed by DMA issues.
            def xpiece(ch, half, ring):
                flat = xT_sb[ch].rearrange("p tb c t -> p (tb c t)")
                ring.dma_start(
                    out=flat[:, 1024 * half : 1024 * (half + 1)],
                    in_=xTc[ch][:, 1024 * half : 1024 * (half + 1)],
                )

            nc.sync.dma_start(out=wq_sb.rearrange("p c n -> p (c n)"), in_=wq[:])
            nc.scalar.dma_start(out=wk_sb.rearrange("p c n -> p (c n)"), in_=wk[:])
            xpiece(0, 0, nc.sync)
            xpiece(0, 1, nc.scalar)
            nc.gpsimd.dma_start(out=wv_sb.rearrange("p c n -> p (c n)"), in_=wv[:])
            for ch in range(1, 4):
                xpiece(ch, 0, nc.sync)
                xpiece(ch, 1, nc.gpsimd)
            nc.gpsimd.dma_start(out=wp_sb.rearrange("p c n -> p (c n)"), in_=wp[:])

            # ---- ACT exp table preload on junk data (hides ~2.7us load) ----
            nc.scalar.activation(out=junk_sb[:], in_=warm_sb[:, 0:P], func=FT.Exp)

            # ---- PE warm-ups: cover the input DMA landing (~4us) ----
            for wi in range(24):
                ps = stg_pool.tile([P, 512], F32, tag="stg", name=f"warm{wi}")
                nc.tensor.matmul(
                    ps[:, 0:256],
                    lhsT=warm_sb[:, 0:P],
                    rhs=warm_sb[:, 0:256],
                    start=True,
                    stop=True,
                )

            # ---- staging / projection fillers ----
            def qk_chunk(w_sb, dest, pt, ch, nm):
                ps = stg_pool.tile([P, 512], F32, tag="stg", name=nm)
                for c in range(4):
                    nc.tensor.matmul(
                        ps[:],
                        lhsT=w_sb[:, c, 128 * pt : 128 * (pt + 1)],
                        rhs=xT_sb[ch][:, :, c, :],
                        start=(c == 0),
                        stop=(c == 3),
                    )
                nc.vector.tensor_copy(dest[:, pt, 512 * ch : 512 * (ch + 1)], ps[:])

            def v_block(tb):
                ps = stg_pool.tile([P, 256], F32, tag="stg", name=f"v_{tb}")
                for c in range(4):
                    nc.tensor.matmul(
                        ps[:],
                        lhsT=xT_sb[tb // 4][:, tb % 4, c, :],
                        rhs=wv_sb[:, c, :],
                        start=(c == 0),
                        stop=(c == 3),
                    )
                nc.vector.tensor_copy(
                    v_sb[:, tb, :].rearrange("p (h c) -> p h c", c=P)[:, :, 0:K],
                    ps.rearrange("p (h c) -> p h c", c=K),
                )

            proj_done = {w: 0 for w in range(4)}

            def proj_m(w, m):
                ps = stg_pool.tile([P, 512], F32, tag="stg", name=f"proj_{w}_{m}")
                for c in range(2):
                    nc.tensor.matmul(
                        ps[:],
                        lhsT=wp_sb[:, c, 128 * m : 128 * (m + 1)],
                        rhs=zn_sb[:, c, 512 * w : 512 * (w + 1)],
                        start=(c == 0),
                        stop=(c == 1),
                    )
                nc.vector.tensor_copy(yT_sb[:, m, 512 * w : 512 * (w + 1)], ps[:])
                proj_done[w] += 1
                if proj_done[w] == 4:
                    nc.sync.dma_start(
                        out=yT_dram[:, :, 512 * w : 512 * (w + 1)],
                        in_=yT_sb[:, :, 512 * w : 512 * (w + 1)],
                    )

            # ---- attention block stream ----
            blocks = []
            seg_start, seg_last = {}, {}
            for pc in range(4):
                for pair in (0, 1):
                    seg_start[(pair, pc)] = len(blocks)
                    for i in range(4 * pc + 4):
                        blocks.append((pair, pc, i))
                    seg_last[(pair, pc)] = len(blocks) - 1
            NB = len(blocks)

            sc_tiles = {}
            ex_tiles = {}
            outs = {}

            def geom(t):
                pair, pc, i = blocks[t]
                qlo = 512 * pc
                wlo = max(P * i, qlo)
                plen = qlo + 512 - wlo
                return pair, pc, i, qlo, wlo, plen

            def emit_qk(t):
                pair, pc, i, qlo, wlo, plen = geom(t)
                sct = sc_pool.tile([P, 1024], F32, tag="sc", name=f"sc{t}")
                sc_tiles[t] = sct
                for hh in range(2):
                    po = 64 * hh
                    nc.tensor.matmul(
                        sct[:, 512 * hh : 512 * hh + plen],
                        lhsT=kT_sb[po : po + 64, pair, P * i : P * (i + 1)],
                        rhs=qT_sb[po : po + 64, pair, wlo : wlo + plen],
                        start=True,
                        stop=True,
                    )

            def emit_exp(t):
                pair, pc, i, qlo, wlo, plen = geom(t)
                sct = sc_tiles[t]
                ex = ex_pool.tile([P, 1024], F16, tag="ex", name=f"ex{t}")
                ex_tiles[t] = ex
                nc.scalar.activation(
                    out=ex.rearrange("p (b c) -> p b c", b=2)[:, :, :plen],
                    in_=sct.rearrange("p (b c) -> p b c", b=2)[:, :, :plen],
                    func=FT.Exp,
                )

            def emit_mask(t):
                pair, pc, i, qlo, wlo, plen = geom(t)
                if P * i >= qlo:  # diagonal block: mask first 128 cols
                    ex = ex_tiles[t]
                    exm = ex.rearrange("p (b c) -> p b c", b=2)[:, :, 0:P]
                    nc.vector.tensor_tensor(exm, exm, trimask[:], OP.mult)

            def emit_av(t):
                pair, pc, i, qlo, wlo, plen = geom(t)
                ex = ex_tiles.pop(t)
                sc_tiles.pop(t)
                n_i = 4 * pc + 4
                if (pair, pc) == (1, 3):
                    # split accumulator into col halves in separate banks so
                    # the A half (final after i=13, by causality) normalizes
                    # and projects under the tail of the exp stream
                    if i == 0:
                        outs[(1, 3)] = [
                            outs_pool.tile([P, 256], F32, tag="outs", name="oA_h0"),
                            outs_pool.tile([P, 256], F32, tag="outs", name="oA_h1"),
                            stg_pool.tile([P, 256], F32, tag="stg", name="oB_h0"),
                            stg_pool.tile([P, 256], F32, tag="stg", name="oB_h1"),
                        ]
                    oA0, oA1, oB0, oB1 = outs[(1, 3)]
                    rel = wlo - qlo
                    for hh in range(2):
                        h = 2 + hh
                        oA, oB = (oA0, oB0) if hh == 0 else (oA1, oB1)
                        if rel < 256:
                            nc.tensor.matmul(
                                oA[:, rel:256],
                                lhsT=v_sb[:, i, h * P : (h + 1) * P],
                                rhs=ex[:, 512 * hh : 512 * hh + (256 - rel)],
                                start=(i == 0),
                                stop=(i == 13),
                            )
                        bs = max(rel, 256)
                        nc.tensor.matmul(
                            oB[:, bs - 256 : 256],
                            lhsT=v_sb[:, i, h * P : (h + 1) * P],
                            rhs=ex[:, 512 * hh + bs - rel : 512 * hh + plen],
                            start=(i == 0),
                            stop=(i == n_i - 1),
                        )
                    return
                if i == 0:
                    outs[(pair, pc)] = [
                        outs_pool.tile(
                            [P, 512], F32, tag="outs", name=f"o{pair}_{pc}_{hh}"
                        )
                        for hh in range(2)
                    ]
                for hh in range(2):
                    h = 2 * pair + hh
                    nc.tensor.matmul(
                        outs[(pair, pc)][hh][:, wlo - qlo : wlo - qlo + plen],
                        lhsT=v_sb[:, i, h * P : (h + 1) * P],
                        rhs=ex[:, 512 * hh : 512 * hh + plen],
                        start=(i == 0),
                        stop=(i == n_i - 1),
                    )

            oc_tiles = {}

            def emit_oc(pair, pc):
                oc = oc_pool.tile(
                    [K + 1, 2, 512], F32, tag="oc", name=f"oc{pair}_{pc}"
                )
                oc_tiles[(pair, pc)] = oc
                for hh in range(2):
                    # copy out of PSUM immediately: frees the bank for next segment
                    nc.vector.tensor_copy(
                        oc[:, hh, :], outs[(pair, pc)][hh][0 : K + 1, :]
                    )

            def emit_norm_rest(pair, pc):
                qlo = 512 * pc
                oc = oc_tiles.pop((pair, pc))
                den = nrm_pool.tile([1, 1024], F32, tag="den", name=f"den{pair}_{pc}")
                nc.vector.tensor_copy(
                    den.rearrange("o (b c) -> o b c", b=2), oc[K : K + 1, :, :]
                )
                rec = nrm_pool.tile([1, 1024], F32, tag="rec", name=f"rec{pair}_{pc}")
                nc.vector.reciprocal_approx_fast(out=rec[0:1, :], in_=den[0:1, :])
                recb = nrm_pool.tile([64, 1024], F32, tag="recb", name=f"recb{pair}_{pc}")
                nc.gpsimd.partition_broadcast(recb[:], rec[0:1, :])
                for hh in range(2):
                    po = 64 * hh
                    nc.vector.tensor_tensor(
                        zn_sb[po : po + 64, pair, qlo : qlo + 512],
                        oc[0:K, hh, :],
                        recb[:, 512 * hh : 512 * (hh + 1)],
                        OP.mult,
                    )

            dn13 = [
                pp.tile([1, 512], F32, tag="dn13a", name="dn13a"),
                pp.tile([1, 512], F32, tag="dn13b", name="dn13b"),
            ]

            def emit_dn13(half):
                o_h0, o_h1 = outs[(1, 3)][2 * half : 2 * half + 2]
                nc.vector.tensor_copy(dn13[half][0:1, 0:256], o_h0[K : K + 1, :])
                nc.vector.tensor_copy(dn13[half][0:1, 256:512], o_h1[K : K + 1, :])

            rb13 = [
                pp.tile([64, 512], F32, tag="rb13a", name="rb13a"),
                pp.tile([64, 512], F32, tag="rb13b", name="rb13b"),
            ]

            def emit_rec13(half):
                rc = pp.tile([1, 512], F32, tag=f"rc13{half}", name=f"rc13{half}")
                nc.vector.reciprocal_approx_fast(out=rc[0:1, :], in_=dn13[half][0:1, :])
                nc.gpsimd.partition_broadcast(rb13[half][:], rc[0:1, :])

            def emit_mults13(half):
                o_h0, o_h1 = outs[(1, 3)][2 * half : 2 * half + 2]
                lo = 1536 + 256 * half
                for hh in range(2):
                    po = 64 * hh
                    nc.vector.tensor_tensor(
                        zn_sb[po : po + 64, 1, lo : lo + 256],
                        (o_h0 if hh == 0 else o_h1)[0:K, :],
                        rb13[half][:, 256 * hh : 256 * (hh + 1)],
                        OP.mult,
                    )

            def emit_proj3_half(half, pools):
                lo = 1536 + 256 * half
                for m in range(4):
                    ps = pools[m % 2].tile(
                        [P, 256], F32, tag=("outs" if pools[m % 2] is outs_pool else "stg"),
                        name=f"pj3_{half}_{m}",
                    )
                    for c in range(2):
                        nc.tensor.matmul(
                            ps[:],
                            lhsT=wp_sb[:, c, 128 * m : 128 * (m + 1)],
                            rhs=zn_sb[:, c, lo : lo + 256],
                            start=(c == 0),
                            stop=(c == 1),
                        )
                    if half == 0 or m % 2 == 1:
                        nc.scalar.copy(yT_sb[:, m, lo : lo + 256], ps[:])
                    else:
                        nc.vector.tensor_copy(yT_sb[:, m, lo : lo + 256], ps[:])
                    nc.sync.dma_start(
                        out=yT_dram[:, m, lo : lo + 256], in_=yT_sb[:, m, lo : lo + 256]
                    )

            # ---- filler schedule: (deadline, seq, cost_ns, fn) ----
            pending = []
            seqno = [0]

            def add_unit(fn, cost, deadline):
                pending.append([deadline, seqno[0], cost, fn])
                seqno[0] += 1
                pending.sort()

            def mk_kq(w_sb, dest, pt, ch, nm):
                return lambda: qk_chunk(w_sb, dest, pt, ch, nm)

            for pc in range(4):
                for pair in (0, 1):
                    if (pair, pc) == (0, 0):
                        continue  # staged in prologue
                    lead = 2 if pc == 0 else 6
                    dl = max(0, seg_start[(pair, pc)] - lead)
                    add_unit(mk_kq(wk_sb, kT_sb, pair, pc, f"k{pair}{pc}"), 1010, dl)
                    add_unit(mk_kq(wq_sb, qT_sb, pair, pc, f"q{pair}{pc}"), 1010, dl + 1)
            for tb in range(3, NSB):
                # v(tb) first consumed by AV of block (0, tb//4, i=tb)
                dl = seg_start[(0, tb // 4)] + (tb - 4 * (tb // 4))
                add_unit((lambda b: lambda: v_block(b))(tb), 590, dl - 1)

            def act_cost(t):
                plen = geom(t)[5]
                return (2 * plen + 420) / 1.2

            def attn_pe_cost(t):
                plen = geom(t)[5]
                return plen / 2.4 + 40 + 2 * plen / 2.4 + 80

            # ---- prologue staging for segment (0,0) ----
            qk_chunk(wk_sb, kT_sb, 0, 0, "k00")
            qk_chunk(wq_sb, qT_sb, 0, 0, "q00")
            v_block(0)

            # ---- main pipelined stream ----
            emit_qk(0)
            emit_exp(0)
            emit_mask(0)
            v_block(1)
            v_block(2)
            act_ns = 1200.0
            pe_ns = 0.0
            for t in range(NB):
                act_ns += act_cost(t)
                pe_ns += attn_pe_cost(t)
                # mandatory (deadline) fillers
                while pending and pending[0][0] <= t:
                    _, _, cost, fn = pending.pop(0)
                    fn()
                    pe_ns += cost
                if t + 1 < NB:
                    emit_qk(t + 1)
                    emit_exp(t + 1)
                emit_av(t)
                pair, pc, i = blocks[t]
                if (pair, pc, i) == (1, 3, 13):
                    emit_dn13(0)
                    emit_rec13(0)  # A half final by causality
                if t == seg_last[(pair, pc)]:
                    if not (pair == 1 and pc == 3):
                        emit_oc(pair, pc)
                        ndl = [26, 32, 46, 60][pc] + 2 * pair
                        add_unit(
                            (lambda pr, w: lambda: emit_norm_rest(pr, w))(pair, pc),
                            0,
                            ndl,
                        )
                    if pair == 1 and pc < 3:
                        for m in range(4):
                            dl = [36, 44, 56][pc] + 3 * m
                            add_unit(
                                (lambda w, mm: lambda: proj_m(w, mm))(pc, m),
                                550,
                                dl,
                            )
                # opportunistic fillers into ACT slack
                while pending and pe_ns + pending[0][2] <= act_ns - 300:
                    _, _, cost, fn = pending.pop(0)
                    fn()
                    pe_ns += cost
                if t + 1 < NB:
                    emit_mask(t + 1)

            # ---- tail: B-half critical chain first in every engine FIFO,
            # A-half mults/projection fill around it ----
            for ent in pending:
                ent[3]()
            pending = []
            emit_dn13(1)
            emit_rec13(1)
            emit_mults13(0)
            emit_proj3_half(0, [outs_pool, outs_pool])  # A: outs slots free
            emit_mults13(1)
            emit_proj3_half(1, [stg_pool, stg_pool])

    nc.finalize()
    return nc


_NC = None


def _get_nc():
    global _NC
    if _NC is None:
        _NC = build_nc()
    return _NC


def make_in_maps(x, w_qkv, w_proj):
    x = np.asarray(x, dtype=np.float32)
    w_qkv = np.asarray(w_qkv, dtype=np.float32)
    w_proj = np.asarray(w_proj, dtype=np.float32)

    def pmajor(w, groups):  # [groups*128, n] -> [128, groups*n] partition-major
        g, n = groups, w.shape[1]
        return np.ascontiguousarray(
            w.reshape(g, 128, n).transpose(1, 0, 2).reshape(128, g * n)
        )

    in_maps = []
    for c in range(8):
        b = c // 2
        h0 = 4 * (c % 2)
        r = slice(64 * h0, 64 * h0 + 256)
        scale = float(K) ** -0.5
        wqm = ((w_qkv[0:512][r] * scale).T).astype(np.float16)
        wkm = (w_qkv[512:1024][r].T).astype(np.float16)
        wvm = (w_qkv[1024:1536][r].T).astype(np.float16)
        wpm = (w_proj[:, r].T).astype(np.float16)
        m = {
            "wq": pmajor(wqm, 4),
            "wk": pmajor(wkm, 4),
            "wv": pmajor(wvm, 4),
            "wp": pmajor(wpm, 2),
        }
        # [c, p, ch, tb, t'] -> per chunk [p, tb, c, t'] flattened
        xTf = x[b].T.astype(np.float16).reshape(4, 128, 4, 4, 128)
        for ch in range(4):
            m[f"xT{ch}"] = np.ascontiguousarray(
                xTf[:, :, ch, :, :].transpose(1, 2, 0, 3)
            ).reshape(128, 2048)
        in_maps.append(m)
    return in_maps


def assemble_output(results, b_proj):
    b_proj = np.asarray(b_proj, dtype=np.float32)
    y = np.empty((4, T, D), np.float32)
    for b in range(4):
        # yT dram layout [128, 4, 2048] partition-major -> [512, 2048]
        ya = results[2 * b]["yT"].astype(np.float32).reshape(128, 4, T)
        yb = results[2 * b + 1]["yT"].astype(np.float32).reshape(128, 4, T)
        yTc = (ya + yb).transpose(1, 0, 2).reshape(D, T)
        y[b] = yTc.T + b_proj
    return y


def kernel(x, w_qkv, w_proj, b_proj):
    from concourse.bass_utils import run_bass_kernel_spmd

    nc = _get_nc()
    in_maps = make_in_maps(x, w_qkv, w_proj)
    res = run_bass_kernel_spmd(nc, in_maps, core_ids=list(range(8)))
    return assemble_output(res.results, b_proj)
